# revision 5
# baseline (speedup 1.0000x reference)
"""DeepseekV3 MoE experts layer on 8 Trainium2 NeuronCores.

Strategy: expert-parallel. Host routes token-expert pairs (argsort by expert),
pads each expert's group to a common capacity C, and ships core e:
  - xT_e    [128, 8, C]    x-shard transposed: (p, ko, c) = x_e[c, ko*128+p]
  - gw_e    [11, 128, 8, 128]  gate_w[e] packed per I-column-tile m:
                               (m, p, ko, i) = gate_w[e][ko*128+p, m*128+i]
  - uw_e    same packing of up_w[e]
  - dw_e    [11, 128, 1024]    (k, p, h) = down_w[e][k*128+p, h]
  - out_e   [C, 1024] fp32 result rows (padded rows garbage/zero)
All matmul inputs are float32r (fp32 rounded to 11-bit mantissa, RNE) —
full TensorE rate with ~1e-4 relative accuracy. Host pre-rounds so no
on-chip rounding pass is needed. The per-core kernel is a dense SwiGLU MLP:
  hT = silu(gw.T @ xT) * (uw.T @ xT)   [I on partitions, C free]
  out = hT.T @ dw                      [C on partitions, H free]
Host unsorts and applies router weights.
"""

import math

import numpy as np

H = 1024
I = 1408
E = 8
TOPK = 4
P = 128
KO_H = H // P   # 8
KO_I = I // P   # 11
M_I = I // P    # 11 I-column tiles for gate/up

_CACHE = {}


def _round_f32r(a: np.ndarray) -> np.ndarray:
    """Round fp32 to float32r (11-bit mantissa, round-to-nearest-even)."""
    b = np.ascontiguousarray(a, dtype=np.float32).view(np.uint32)
    b = b + np.uint32(0x7FF) + ((b >> np.uint32(12)) & np.uint32(1))
    b &= np.uint32(0xFFFFF000)
    return b.view(np.float32)


def _plan_capacity(max_cnt: int):
    c0 = max(128, 32 * math.ceil(max_cnt / 32))
    ntiles = math.ceil(c0 / 512)
    base = 32 * math.ceil(c0 / ntiles / 32)
    C = base * ntiles
    return C, [base] * ntiles


def build(C, n_sizes, R=1):
    import concourse.bacc as bacc
    import concourse.mybir as mybir
    import concourse.tile as tile
    from concourse.engine_type import EngineType

    f32 = mybir.dt.float32
    f32r = mybir.dt.float32r
    nc = bacc.Bacc("TRN2", target_bir_lowering=False, debug=False)

    xT_d = nc.dram_tensor("xT", [P, KO_H, C], f32r, kind="ExternalInput").ap()
    gw_d = nc.dram_tensor("gw", [M_I, P, KO_H, P], f32r, kind="ExternalInput").ap()
    uw_d = nc.dram_tensor("uw", [M_I, P, KO_H, P], f32r, kind="ExternalInput").ap()
    dw_d = nc.dram_tensor("dw", [KO_I, P, H], f32r, kind="ExternalInput").ap()
    out_d = nc.dram_tensor("out", [C, H], f32, kind="ExternalOutput").ap()

    MT = math.ceil(C / P)          # token tiles for the down pass
    NH = H // 512                  # 2 H-column tiles for the down pass
    n_off = np.cumsum([0] + n_sizes).tolist()

    def body(nc, tc, sb, ps, sil, outp):
        # SBUF residents
        xT = sb.tile([P, KO_H, C], f32r, tag="xT")
        gw = sb.tile([P, KO_H, I], f32r, tag="gw")
        uw = sb.tile([P, KO_H, I], f32r, tag="uw")
        dw = sb.tile([P, KO_I, H], f32r, tag="dw")
        hT = sb.tile([P, KO_I, C], f32r, tag="hT")

        # input DMAs in consumption order: the first matmuls (m=0) need only
        # the gw/uw m=0 chunks plus xT; dw is consumed ~40us in.
        nc.sync.dma_start(gw[:, :, 0:P], gw_d[0])
        nc.sync.dma_start(uw[:, :, 0:P], uw_d[0])
        for k in range(KO_H):
            nc.sync.dma_start(xT[:, k], xT_d[:, k])
        for m in range(1, M_I):
            nc.sync.dma_start(gw[:, :, m * P:(m + 1) * P], gw_d[m])
            nc.sync.dma_start(uw[:, :, m * P:(m + 1) * P], uw_d[m])
        for k in range(KO_I):
            nc.sync.dma_start(dw[:, k], dw_d[k])

        # gate/up + SwiGLU: psum[I-tile, n] accumulated over H.
        # k outer / n inner so consecutive matmuls share the stationary weights.
        for m in range(M_I):
            msl = slice(m * P, (m + 1) * P)
            pg = ps.tile([P, NTOK, 512], f32, tag="ps", name="pg")
            pu = ps.tile([P, NTOK, 512], f32, tag="ps", name="pu")
            for k in range(KO_H):
                st, sp = (k == 0), (k == KO_H - 1)
                for n, nsz in enumerate(n_sizes):
                    nc.tensor.matmul(pg[:, n, :nsz], gw[:, k, msl],
                                     xT[:, k, n_off[n]:n_off[n + 1]], start=st, stop=sp)
                for n, nsz in enumerate(n_sizes):
                    nc.tensor.matmul(pu[:, n, :nsz], uw[:, k, msl],
                                     xT[:, k, n_off[n]:n_off[n + 1]], start=st, stop=sp)
            for n, nsz in enumerate(n_sizes):
                nsl = slice(n_off[n], n_off[n + 1])
                sg = sil.tile([P, 512], f32, tag="sil", name="sg")[:, :nsz]
                nc.scalar.activation(sg, pg[:, n, :nsz], mybir.ActivationFunctionType.Silu)
                nc.vector.tensor_mul(hT[:, m, nsl], sg, pu[:, n, :nsz])

        # down: psum[token-tile, 512] accumulated over I; hT is stationary
        for mt in range(MT):
            rows = min(P, C - mt * P)
            tsl = slice(mt * P, mt * P + rows)
            pd = ps.tile([P, NTOK, 512], f32, tag="ps", name="pd")
            for k in range(KO_I):
                st, sp = (k == 0), (k == KO_I - 1)
                for n in range(NH):
                    nc.tensor.matmul(pd[:rows, n, :], hT[:, k, tsl],
                                     dw[:, k, n * 512:(n + 1) * 512], start=st, stop=sp)
            for n in range(NH):
                ot = outp.tile([P, 512], f32, tag="o", name="ot")[:rows]
                nc.vector.tensor_copy(ot, pd[:rows, n, :])
                nc.sync.dma_start(out_d[tsl, n * 512:(n + 1) * 512], ot)

    NTOK = max(len(n_sizes), NH)  # psum tile: [P, NTOK, 512] = NTOK banks
    with tile.TileContext(nc) as tc:
        with (
            tc.tile_pool(name="sb", bufs=1) as sb,
            tc.tile_pool(name="ps", bufs=4, space="PSUM") as ps,
            tc.tile_pool(name="sil", bufs=3) as sil,
            tc.tile_pool(name="outp", bufs=4) as outp,
        ):
            if R == 1:
                body(nc, tc, sb, ps, sil, outp)
            else:
                with tc.For_i(0, R, 1, hint_engines=(EngineType.PE,)):
                    body(nc, tc, sb, ps, sil, outp)
    nc.compile()
    return nc


def _route(hidden_states, selected_experts):
    """Host-side dispatch: group token-expert pairs by expert."""
    flat = selected_experts.ravel()
    order = np.argsort(flat, kind="stable")
    counts = np.bincount(flat, minlength=E)
    return flat, order, counts


def _pack_inputs(hidden_states, gate_w, up_w, down_w, order, counts, C):
    starts = np.concatenate([[0], np.cumsum(counts)])
    in_maps = []
    hs_r = _round_f32r(hidden_states)
    for e in range(E):
        pairs = order[starts[e]:starts[e + 1]]
        x_e = np.zeros((C, H), np.float32)
        x_e[: counts[e]] = hs_r[pairs // TOPK]
        # (p, ko, c) = x_e[c, ko*128+p]
        xT = np.ascontiguousarray(x_e.T.reshape(KO_H, P, C).transpose(1, 0, 2))
        gw = np.ascontiguousarray(
            _round_f32r(gate_w[e]).reshape(KO_H, P, M_I, P).transpose(2, 1, 0, 3))
        uw = np.ascontiguousarray(
            _round_f32r(up_w[e]).reshape(KO_H, P, M_I, P).transpose(2, 1, 0, 3))
        dw = np.ascontiguousarray(_round_f32r(down_w[e]).reshape(KO_I, P, H))
        in_maps.append({"xT": xT, "gw": gw, "uw": uw, "dw": dw})
    return in_maps


def _combine(results, router_weights, order, counts, T):
    starts = np.concatenate([[0], np.cumsum(counts)])
    all_down = np.zeros((T * TOPK, H), np.float32)
    for e in range(E):
        pairs = order[starts[e]:starts[e + 1]]
        all_down[pairs] = results[e]["out"][: counts[e]]
    w = router_weights.reshape(T, TOPK, 1).astype(np.float32)
    return (all_down.reshape(T, TOPK, H) * w).sum(axis=1)


def kernel(hidden_states, router_weights, gate_w, up_w, down_w, selected_experts):
    from concourse.bass_utils import run_bass_kernel_spmd

    hidden_states = np.asarray(hidden_states, np.float32)
    router_weights = np.asarray(router_weights, np.float32)
    gate_w = np.asarray(gate_w, np.float32)
    up_w = np.asarray(up_w, np.float32)
    down_w = np.asarray(down_w, np.float32)
    selected_experts = np.asarray(selected_experts)
    T = hidden_states.shape[0]

    flat, order, counts = _route(hidden_states, selected_experts)
    C, n_sizes = _plan_capacity(int(counts.max()))

    key = (C, tuple(n_sizes))
    if key not in _CACHE:
        _CACHE[key] = build(C, n_sizes, R=1)
    nc = _CACHE[key]

    in_maps = _pack_inputs(hidden_states, gate_w, up_w, down_w, order, counts, C)
    res = run_bass_kernel_spmd(nc, in_maps, core_ids=list(range(E)))
    return _combine(res.results, router_weights, order, counts, T)


# revision 6
# speedup vs baseline: 2.6891x; 2.6891x over previous
"""DeepseekV3 MoE experts layer on 8 Trainium2 NeuronCores.

Strategy: expert-parallel. Host routes token-expert pairs (argsort by expert),
truncates each expert's group to a fixed capacity C=512 (overflow pairs — a
handful for balanced routing — are computed on the host in fp32), and ships
core e:
  - xT_e  [128, 8, 512]      x-shard transposed: (p, ko, c) = x_e[c, ko*128+p]
  - gw_e  [11, 128, 8, 128]  gate_w[e] packed per I-column-tile m:
                             (m, p, ko, i) = gate_w[e][ko*128+p, m*128+i]
  - uw_e  same packing of up_w[e]
  - dw_e  [11, 128, 1024]    (k, p, h) = down_w[e][k*128+p, h]
  - out_e [512, 1024] fp32 result rows
Matmul inputs are fp16 (10-bit mantissa, ~bf16 speed on TensorE with the
LDWEIGHTS pull-ahead path and fp32 PSUM accumulation; ~4e-4 rel accuracy).
The per-core kernel is a dense SwiGLU MLP:
  hT = silu(gw.T @ xT) * (uw.T @ xT)   [I on partitions, C free]
  out = hT.T @ dw                      [C on partitions, H free]
Host unsorts and applies router weights.
"""

import math

import numpy as np

H = 1024
I = 1408
E = 8
TOPK = 4
P = 128
KO_H = H // P   # 8
KO_I = I // P   # 11
M_I = I // P    # 11 I-column tiles for gate/up
CAP = 512       # per-expert on-device capacity (overflow handled on host)

_CACHE = {}


def build(R=1, C=CAP):
    import concourse.bacc as bacc
    import concourse.mybir as mybir
    import concourse.tile as tile
    from concourse.engine_type import EngineType

    f32 = mybir.dt.float32
    f16 = mybir.dt.float16
    nc = bacc.Bacc("TRN2", target_bir_lowering=False, debug=False)

    xT_d = nc.dram_tensor("xT", [P, KO_H, C], f16, kind="ExternalInput").ap()
    gw_d = nc.dram_tensor("gw", [M_I, P, KO_H, P], f16, kind="ExternalInput").ap()
    uw_d = nc.dram_tensor("uw", [M_I, P, KO_H, P], f16, kind="ExternalInput").ap()
    dw_d = nc.dram_tensor("dw", [KO_I, P, H], f16, kind="ExternalInput").ap()
    out_d = nc.dram_tensor("out", [C, H], f32, kind="ExternalOutput").ap()

    MT = C // P                    # 4 token tiles for the down pass
    NH = H // 512                  # 2 H-column tiles for the down pass

    def body(nc, tc, sb, ps, sil, outp):
        # SBUF residents
        xT = sb.tile([P, KO_H, C], f16, tag="xT")
        gw = sb.tile([P, KO_H, I], f16, tag="gw")
        uw = sb.tile([P, KO_H, I], f16, tag="uw")
        dw = sb.tile([P, KO_I, H], f16, tag="dw")
        hT = sb.tile([P, KO_I, C], f16, tag="hT")

        # input DMAs in consumption order: the first matmuls (m=0) need only
        # the gw/uw m=0 chunks plus xT; dw is consumed much later.
        nc.sync.dma_start(gw[:, :, 0:P], gw_d[0])
        nc.sync.dma_start(uw[:, :, 0:P], uw_d[0])
        for k in range(KO_H):
            nc.sync.dma_start(xT[:, k], xT_d[:, k])
        for m in range(1, M_I):
            nc.sync.dma_start(gw[:, :, m * P:(m + 1) * P], gw_d[m])
            nc.sync.dma_start(uw[:, :, m * P:(m + 1) * P], uw_d[m])
        for k in range(KO_I):
            nc.sync.dma_start(dw[:, k], dw_d[k])

        # gate/up + SwiGLU: psum[I-tile, C] accumulated over H
        for m in range(M_I):
            msl = slice(m * P, (m + 1) * P)
            pg = ps.tile([P, 512], f32, tag="g", name="pg")
            pu = ps.tile([P, 512], f32, tag="u", name="pu")
            for k in range(KO_H):
                st, sp = (k == 0), (k == KO_H - 1)
                nc.tensor.matmul(pg, gw[:, k, msl], xT[:, k], start=st, stop=sp)
                nc.tensor.matmul(pu, uw[:, k, msl], xT[:, k], start=st, stop=sp)
            sg = sil.tile([P, 512], f32, tag="sil", name="sg")
            nc.scalar.activation(sg, pg, mybir.ActivationFunctionType.Silu)
            nc.vector.tensor_mul(hT[:, m], sg, pu)

        # down: psum[token-tile, 2, 512] accumulated over I; hT is stationary
        for mt in range(MT):
            tsl = slice(mt * P, (mt + 1) * P)
            pd = ps.tile([P, NH, 512], f32, tag="d", name="pd")
            for k in range(KO_I):
                st, sp = (k == 0), (k == KO_I - 1)
                for n in range(NH):
                    nc.tensor.matmul(pd[:, n], hT[:, k, tsl],
                                     dw[:, k, n * 512:(n + 1) * 512], start=st, stop=sp)
            for n in range(NH):
                ot = outp.tile([P, 512], f32, tag="o", name="ot")
                nc.vector.tensor_copy(ot, pd[:, n])
                nc.sync.dma_start(out_d[tsl, n * 512:(n + 1) * 512], ot)

    with tile.TileContext(nc) as tc:
        with (
            tc.tile_pool(name="sb", bufs=1) as sb,
            tc.tile_pool(name="ps", bufs=2, space="PSUM") as ps,
            tc.tile_pool(name="sil", bufs=3) as sil,
            tc.tile_pool(name="outp", bufs=4) as outp,
        ):
            if R == 1:
                body(nc, tc, sb, ps, sil, outp)
            else:
                with tc.For_i(0, R, 1, hint_engines=(EngineType.PE,)):
                    body(nc, tc, sb, ps, sil, outp)
    nc.compile()
    return nc


def _route(hidden_states, selected_experts):
    """Host-side dispatch: group token-expert pairs by expert."""
    flat = selected_experts.ravel()
    order = np.argsort(flat, kind="stable")
    counts = np.bincount(flat, minlength=E)
    return flat, order, counts


def _pack_inputs(hidden_states, gate_w, up_w, down_w, order, counts, C=CAP):
    starts = np.concatenate([[0], np.cumsum(counts)])
    in_maps = []
    hs16 = hidden_states.astype(np.float16)
    for e in range(E):
        cnt = min(int(counts[e]), C)
        pairs = order[starts[e]:starts[e] + cnt]
        x_e = np.zeros((C, H), np.float16)
        x_e[:cnt] = hs16[pairs // TOPK]
        # (p, ko, c) = x_e[c, ko*128+p]
        xT = np.ascontiguousarray(x_e.T.reshape(KO_H, P, C).transpose(1, 0, 2))
        gw = np.ascontiguousarray(
            gate_w[e].astype(np.float16).reshape(KO_H, P, M_I, P).transpose(2, 1, 0, 3))
        uw = np.ascontiguousarray(
            up_w[e].astype(np.float16).reshape(KO_H, P, M_I, P).transpose(2, 1, 0, 3))
        dw = np.ascontiguousarray(down_w[e].astype(np.float16).reshape(KO_I, P, H))
        in_maps.append({"xT": xT, "gw": gw, "uw": uw, "dw": dw})
    return in_maps


def _host_expert(x, gate_w_e, up_w_e, down_w_e):
    """fp32 host fallback for capacity-overflow rows."""
    g = x @ gate_w_e
    u = x @ up_w_e
    h = (g / (1.0 + np.exp(-g))) * u
    return h @ down_w_e


def _combine(results, hidden_states, router_weights, gate_w, up_w, down_w,
             order, counts, T, C=CAP):
    starts = np.concatenate([[0], np.cumsum(counts)])
    all_down = np.zeros((T * TOPK, H), np.float32)
    for e in range(E):
        cnt = min(int(counts[e]), C)
        pairs = order[starts[e]:starts[e] + cnt]
        all_down[pairs] = results[e]["out"][:cnt]
        if counts[e] > C:
            over = order[starts[e] + C:starts[e + 1]]
            x = hidden_states[over // TOPK].astype(np.float32)
            all_down[over] = _host_expert(x, gate_w[e], up_w[e], down_w[e])
    w = router_weights.reshape(T, TOPK, 1).astype(np.float32)
    return (all_down.reshape(T, TOPK, H) * w).sum(axis=1)


def kernel(hidden_states, router_weights, gate_w, up_w, down_w, selected_experts):
    from concourse.bass_utils import run_bass_kernel_spmd

    hidden_states = np.asarray(hidden_states, np.float32)
    router_weights = np.asarray(router_weights, np.float32)
    gate_w = np.asarray(gate_w, np.float32)
    up_w = np.asarray(up_w, np.float32)
    down_w = np.asarray(down_w, np.float32)
    selected_experts = np.asarray(selected_experts)
    T = hidden_states.shape[0]

    flat, order, counts = _route(hidden_states, selected_experts)

    if "nc" not in _CACHE:
        _CACHE["nc"] = build(R=1)
    nc = _CACHE["nc"]

    in_maps = _pack_inputs(hidden_states, gate_w, up_w, down_w, order, counts)
    res = run_bass_kernel_spmd(nc, in_maps, core_ids=list(range(E)))
    return _combine(res.results, hidden_states, router_weights, gate_w, up_w,
                    down_w, order, counts, T)
